# revision 34
# baseline (speedup 1.0000x reference)
"""Distributed Trainium2 Bass kernel for nn_AnomalyGNN (3x GCNConv + per-graph MLP).

Strategy (8 NeuronCores, node/graph parallel):
  - Nodes block-sharded: core c owns rows [c*NL, (c+1)*NL), NL = 18750.
  - Edges sharded by destination; per core they are grouped by
    (dst miniblock of 32 nodes, source segment of 30000 nodes) and chunked
    into 128-edge chunks (row-0 padding; the chunk structure is the
    per-(mb,seg) max over cores so all 8 cores share one SPMD graph).
  - Per layer: t = h @ W for the local shard (feature-major, W stationary on
    TensorE), transposed to node-major, written to an internal DRAM shard and
    AllGathered into the full [N, 128] table. The node-major transpose blocks
    are also kept in SBUF scaled by dinv^2 (ts) for the self-loop term.
  - Message passing: MoE dma_gather (int16 idx into one of 5 segment
    sub-tables) pulls table rows; aggregation is
    agg^T[f, dst] += G_chunk^T @ S_chunk on TensorE, where S is the
    host-built [128 edges x 32 dst] matrix carrying the D^-1/2 A D^-1/2
    coefficients. Gather calls are split into <=16-chunk pieces rotating
    over the 4 SWDGE queues so descriptor generation uses all 8 GpSimd Q7
    cores; deep per-piece tile buffering keeps descgen running during the
    PSUM sweeps. Each 128-node PSUM group opens with one wide self-loop
    matmul (node-major table block DMA'd back from tb_sh, scaled by dinv^2),
    rhs=identity, then accumulates the edge chunks; epilogue relu(agg + b)
    on ScalarE emits the next feature-major h.
  - Layer l+1's table production (W-matmul, transpose, shard write and
    sub-AllGathers) is issued interleaved into layer l's PSUM sweeps so the
    collectives complete before the layer boundary; the trailing per-graph
    MLP runs in two halves, the first interleaved under layer 2's sweeps.

Compute in bf16 (f32 PSUM accumulation), I/O in f32.
"""

import numpy as np
import ml_dtypes

# ---------------------------------------------------------------- constants
N = 150000        # nodes
F = 128           # in/hidden channels
P3 = 3            # num_protocols
NC = 8            # cores
NL = N // NC      # nodes per core = 18750
GPC = NL // P3    # graphs per core = 6250
MB = 32           # dst nodes per miniblock
K = 128           # edges per chunk (= matmul contraction)
N_MB = (NL + MB - 1) // MB     # 586
SEG = 30000       # source-segment size (int16-addressable)
SUBW = 3750       # per-rank slice width per sub-AllGather; SEG = NC * SUBW
N_SEG = NL // SUBW  # 5
SG = 32           # miniblocks per supergroup (gather-call granularity)
N_SG = (N_MB + SG - 1) // SG   # 19
N_GRP = (N_MB + 3) // 4        # 147 psum groups of 128 dst nodes
FO = [128, 128, 64]            # per-layer output widths (table always 128 wide)
PIECE = 32        # max chunks per gather sub-call
import os as _os
NQ = int(_os.environ.get("KERNEL_NQ", "4"))

BF16 = ml_dtypes.bfloat16


# ------------------------------------------------------------ preprocessing
def _preprocess(edge_index):
    src = np.asarray(edge_index[0], np.int64)
    dst = np.asarray(edge_index[1], np.int64)
    deg = np.bincount(dst, minlength=N).astype(np.float64) + 1.0
    dinv = (1.0 / np.sqrt(deg)).astype(np.float32)
    norm = (dinv[src] * dinv[dst]).astype(np.float32)

    core = dst // NL
    NKEY = N_MB * N_SEG
    percore = []
    counts = np.zeros((NC, NKEY), dtype=np.int64)
    for c in range(NC):
        m = core == c
        lsrc = src[m]
        ldst = dst[m] - c * NL
        lnorm = norm[m]
        key = (ldst // MB) * N_SEG + (lsrc % NL) // SUBW
        order = np.argsort(key, kind="stable")
        lsrc, ldst, lnorm, key = lsrc[order], ldst[order], lnorm[order], key[order]
        counts[c] = np.bincount(key, minlength=NKEY)
        percore.append((lsrc, ldst, lnorm, key))

    cpm = (counts.max(axis=0) + K - 1) // K        # [NKEY] chunks per (mb, seg)
    cpm2 = cpm.reshape(N_MB, N_SEG)
    assert (cpm2.sum(axis=1) >= 1).all()

    # global chunk column order: (sg, s, mb, k)
    chunk_col = np.zeros((N_MB, N_SEG), np.int64)
    calls = []          # (sg, s, col_base, n_chunks)
    col = 0
    for sg in range(N_SG):
        mb0, mb1 = sg * SG, min((sg + 1) * SG, N_MB)
        for s in range(N_SEG):
            base = col
            for mb in range(mb0, mb1):
                chunk_col[mb, s] = col
                col += cpm2[mb, s]
            if col > base:
                calls.append((sg, s, base, col - base))
    totch = col

    idx16 = np.zeros((NC, 128, totch * 8), np.int16)
    S_hbm = np.zeros((NC, 128, totch * MB), BF16)
    for c in range(NC):
        lsrc, ldst, lnorm, key = percore[c]
        starts = np.zeros(NKEY, np.int64)
        starts[1:] = np.cumsum(counts[c])[:-1]
        pos = np.arange(len(lsrc)) - starts[key]
        colg = chunk_col[key // N_SEG, key % N_SEG] + pos // K
        lane = pos % K
        idxflat = np.zeros((totch, K), np.int64)      # default row 0 (padding)
        idxflat[colg, lane] = (lsrc // NL) * SUBW + (lsrc % NL) % SUBW
        S_all = np.zeros((totch, K, MB), np.float32)
        slot = ldst % MB
        S_all.reshape(-1)[(colg * K + lane) * MB + slot] = lnorm
        for sg, s, base, nch in calls:
            v = idxflat[base : base + nch].reshape(-1)
            a = v.reshape(nch * 8, 16).T.astype(np.int16)
            idx16[c, :, base * 8 : (base + nch) * 8] = np.tile(a, (8, 1))
        S_hbm[c] = S_all.transpose(1, 0, 2).reshape(128, totch * MB).astype(BF16)

    # d2 node-major per core: d2nm[p, g] = dinv^2 at local node g*128+p
    d2nm = np.zeros((NC, 128, N_GRP), BF16)
    for c in range(NC):
        d2 = (dinv[c * NL : (c + 1) * NL].astype(np.float64) ** 2).astype(np.float32)
        pad = np.zeros(N_GRP * 128, np.float32)
        pad[:NL] = d2
        d2nm[c] = pad.reshape(N_GRP, 128).T.astype(BF16)

    return dict(
        cpm2=cpm2,
        chunk_col=chunk_col,
        calls=calls,
        totch=totch,
        idx16=idx16,
        S_hbm=S_hbm,
        d2nm=d2nm,
    )


def _last_seg(cpm2, mb):
    for s in range(N_SEG - 1, -1, -1):
        if cpm2[mb, s] > 0:
            return s
    return 0


# ------------------------------------------------------------ graph builder
def _build(struct):
    import concourse.bass as bass
    import concourse.tile as tile
    from concourse import bacc, mybir
    from concourse.masks import make_identity

    bf = mybir.dt.bfloat16
    f32 = mybir.dt.float32
    i16 = mybir.dt.int16
    AF = mybir.ActivationFunctionType
    ALU = mybir.AluOpType

    cpm2 = struct["cpm2"]
    chunk_col = struct["chunk_col"]
    calls = struct["calls"]
    totch = struct["totch"]

    nc = bacc.Bacc(
        "TRN2", target_bir_lowering=False, debug=False, num_devices=NC,
        num_swdge_queues=NQ,
    )

    # ---- I/O
    xT = nc.dram_tensor("xT", [F, NL], bf, kind="ExternalInput")
    Wt = [
        nc.dram_tensor("W1", [F, 128], bf, kind="ExternalInput"),
        nc.dram_tensor("W2", [128, 128], bf, kind="ExternalInput"),
        nc.dram_tensor("W3", [128, 64], bf, kind="ExternalInput"),
    ]
    bt = [
        nc.dram_tensor("b1", [128, 1], f32, kind="ExternalInput"),
        nc.dram_tensor("b2", [128, 1], f32, kind="ExternalInput"),
        nc.dram_tensor("b3", [64, 1], f32, kind="ExternalInput"),
    ]
    Wc1a = nc.dram_tensor("Wc1a", [128, 128], bf, kind="ExternalInput")
    Wc1b = nc.dram_tensor("Wc1b", [64, 128], bf, kind="ExternalInput")
    Wc2 = nc.dram_tensor("Wc2", [128, 64], bf, kind="ExternalInput")
    Wc3d = nc.dram_tensor("Wc3d", [64, 2], bf, kind="ExternalInput")
    bc1 = nc.dram_tensor("bc1", [128, 1], f32, kind="ExternalInput")
    bc2 = nc.dram_tensor("bc2", [64, 1], f32, kind="ExternalInput")
    bc3dd = nc.dram_tensor("bc3dd", [2, 1], f32, kind="ExternalInput")
    idx_h = nc.dram_tensor("idx", [128, totch * 8], i16, kind="ExternalInput")
    S_h = nc.dram_tensor("S", [128, totch * MB], bf, kind="ExternalInput")
    d2_h = nc.dram_tensor("d2nm", [128, N_GRP], bf, kind="ExternalInput")
    out_t = nc.dram_tensor("out", [GPC, 2], f32, kind="ExternalOutput")

    # ---- internal DRAM (tables always 128 wide)
    tb_sh = [nc.dram_tensor(f"tbsh{l}", [NL, 128], bf) for l in range(3)]
    tbl = [
        [
            nc.dram_tensor(f"tbl{l}s{k}", [SEG, 128], bf, addr_space="Shared")
            for k in range(N_SEG)
        ]
        for l in range(3)
    ]
    RG = [list(range(NC))]

    qrr = [0]   # gather queue round-robin (DMASW lane i%8 <-> queue i%NQ)

    def next_q():
        q = qrr[0]
        qrr[0] = (q + 1) % NQ
        return q

    with tile.TileContext(nc) as tc:
        from contextlib import ExitStack

        with ExitStack() as _es:
            p_const = _es.enter_context(tc.tile_pool(name="const", bufs=1))
            p_h = _es.enter_context(tc.tile_pool(name="h", bufs=1))
            p_z = _es.enter_context(tc.tile_pool(name="z", bufs=1))
            p_mlp = _es.enter_context(tc.tile_pool(name="mlp", bufs=2))
            p_e = _es.enter_context(tc.tile_pool(name="e", bufs=7))
            p_rhs = _es.enter_context(tc.tile_pool(name="rhs", bufs=3))
            p_t = _es.enter_context(tc.tile_pool(name="tseg", bufs=3))
            p_u = _es.enter_context(tc.tile_pool(name="ust", bufs=3))
            p_sf = _es.enter_context(tc.tile_pool(name="self", bufs=4))
            p_i = _es.enter_context(tc.tile_pool(name="idx", bufs=10))
            p_s = _es.enter_context(tc.tile_pool(name="smat", bufs=7))
            p_g = _es.enter_context(tc.tile_pool(name="gat", bufs=10))
            p_pw = _es.enter_context(tc.tile_pool(name="pw", bufs=2, space="PSUM"))
            p_ptr = _es.enter_context(tc.tile_pool(name="ptr", bufs=2, space="PSUM"))
            p_agg = _es.enter_context(tc.tile_pool(name="pagg", bufs=4, space="PSUM"))
            # ---------------- persistent constants
            ident = p_const.tile([128, 128], bf, tag="ident")
            make_identity(nc, ident[:])
            w_tiles, b_tiles = [], []
            for l in range(3):
                wt = p_const.tile([128, FO[l]], bf, tag=f"w{l}")
                nc.sync.dma_start(wt[:], Wt[l][:, :])
                w_tiles.append(wt)
                btl = p_const.tile([FO[l], 1], f32, tag=f"b{l}")
                nc.sync.dma_start(btl[:], bt[l][:, :])
                b_tiles.append(btl)
            wc1a = p_const.tile([128, 128], bf, tag="wc1a")
            nc.sync.dma_start(wc1a[:], Wc1a[:, :])
            wc1b = p_const.tile([64, 128], bf, tag="wc1b")
            nc.sync.dma_start(wc1b[:], Wc1b[:, :])
            wc2 = p_const.tile([128, 64], bf, tag="wc2")
            nc.sync.dma_start(wc2[:], Wc2[:, :])
            wc3d = p_const.tile([64, 2], bf, tag="wc3d")
            nc.sync.dma_start(wc3d[:], Wc3d[:, :])
            bc1t = p_const.tile([128, 1], f32, tag="bc1")
            nc.sync.dma_start(bc1t[:], bc1[:, :])
            bc2t = p_const.tile([64, 1], f32, tag="bc2")
            nc.sync.dma_start(bc2t[:], bc2[:, :])
            bdd = p_const.tile([2, 1], f32, tag="bdd")
            nc.sync.dma_start(bdd[:], bc3dd[:, :])
            d2t = p_const.tile([128, N_GRP], bf, tag="d2")
            nc.sync.dma_start(d2t[:], d2_h[:, :])

            h_tiles = [None, None, None]
            ag_state = [0, 0, 0]   # next sub-AllGather per layer
            AG_DELAY = 10          # groups between AG readiness and issue

            def fire_ag(l, s):
                nc.gpsimd.collective_compute(
                    "AllGather",
                    ALU.bypass,
                    replica_groups=RG,
                    ins=[tb_sh[l][s * SUBW : (s + 1) * SUBW, :].opt()],
                    outs=[tbl[l][s].ap().opt()],
                )

            # ---- B phase: one 512-col slice of t = h @ W for layer l.
            # Emits matmul, transpose, shard write; fires (or queues into
            # ag_sink) any sub-AllGathers whose source slice completed. The
            # self-loop term is re-fetched from tb_sh per PSUM group during
            # the sweep (no persistent ts).
            def emit_B_slice(l, c0, ag_sink=None):
                fi = F if l == 0 else FO[l - 1]
                fo = FO[l]
                w = min(512, NL - c0)
                if l == 0:
                    rhs_t = p_rhs.tile([F, 512], bf)
                    nc.sync.dma_start(rhs_t[:, :w], xT[:, c0 : c0 + w])
                    rhs_ap = rhs_t[:, :w]
                else:
                    rhs_ap = h_tiles[l - 1][:fi, c0 : c0 + w]
                pt = p_pw.tile([128, 512], f32, tag="pw")
                nc.tensor.matmul(
                    pt[:fo, :w], lhsT=w_tiles[l][:fi, :fo], rhs=rhs_ap,
                    start=True, stop=True,
                )
                tseg = p_t.tile([128, 512], bf)
                nc.vector.tensor_copy(tseg[:fo, :w], pt[:fo, :w])
                ust = p_u.tile([128, 4, 128], bf)
                nbl = (w + 127) // 128
                for sb in range(nbl):
                    sw = min(128, w - sb * 128)
                    ptt = p_ptr.tile([128, 128], bf, tag="ptr")
                    nc.tensor.transpose(
                        ptt[:sw, :fo],
                        tseg[:fo, sb * 128 : sb * 128 + sw],
                        ident[:fo, :fo],
                    )
                    nc.vector.tensor_copy(ust[:sw, sb, :fo], ptt[:sw, :fo])
                nfull = w // 128
                if nfull:
                    dst_ap = tb_sh[l][c0 : c0 + nfull * 128, :].rearrange(
                        "(j p) f -> p j f", p=128
                    )
                    nc.sync.dma_start(dst_ap, ust[:, :nfull, :])
                if w % 128:
                    tw = w % 128
                    nc.sync.dma_start(
                        tb_sh[l][c0 + nfull * 128 : c0 + w, :],
                        ust[:tw, nfull, :],
                    )
                while ag_state[l] < N_SEG and c0 + w >= (ag_state[l] + 1) * SUBW:
                    s = ag_state[l]
                    if ag_sink is None:
                        fire_ag(l, s)
                    else:
                        ag_sink.append(s)
                    ag_state[l] += 1

            # ---- MLP parts: graph ranges fired as their h columns complete.
            # 2-class log_softmax via softplus: out0 = -softplus(d),
            # out1 = -softplus(-d), d = z2.(wc3[:,1]-wc3[:,0]) + (bc3[1]-bc3[0])
            PB = [0, 3125, 4688, GPC]
            MLP_THRESH = [73, 109]     # sweep group gating parts 0 and 1
            PWID = 3125

            def emit_mlp_part(p):
                h_cur = h_tiles[2]
                gl, gr = PB[p], PB[p + 1]
                wp = gr - gl
                zA = p_z.tile([128, PWID], bf, tag="zA")
                zB = p_z.tile([64, PWID], bf, tag="zB")
                z1 = p_z.tile([128, PWID], bf, tag="z1")
                z2 = p_z.tile([64, PWID], bf, tag="z2")
                nc.vector.tensor_copy(zA[0:64, :wp], h_cur[:64, 3 * gl + 0 : 3 * gr : 3])
                nc.vector.tensor_copy(zA[64:128, :wp], h_cur[:64, 3 * gl + 1 : 3 * gr : 3])
                nc.vector.tensor_copy(zB[:, :wp], h_cur[:64, 3 * gl + 2 : 3 * gr : 3])
                for c0 in range(0, wp, 512):
                    w = min(512, wp - c0)
                    pz = p_pw.tile([128, 512], f32, tag="pw")
                    nc.tensor.matmul(
                        pz[:, :w], lhsT=wc1a[:], rhs=zA[:, c0 : c0 + w],
                        start=True, stop=False,
                    )
                    nc.tensor.matmul(
                        pz[:, :w], lhsT=wc1b[:], rhs=zB[:, c0 : c0 + w],
                        start=False, stop=True,
                    )
                    nc.scalar.activation(
                        z1[:, c0 : c0 + w], pz[:, :w], AF.Relu, bias=bc1t[:, :]
                    )
                for c0 in range(0, wp, 512):
                    w = min(512, wp - c0)
                    pz = p_pw.tile([128, 512], f32, tag="pw")
                    nc.tensor.matmul(
                        pz[:64, :w], lhsT=wc2[:], rhs=z1[:, c0 : c0 + w],
                        start=True, stop=True,
                    )
                    nc.scalar.activation(
                        z2[:, c0 : c0 + w], pz[:64, :w], AF.Relu, bias=bc2t[:, :]
                    )
                # log_softmax via -ln(1+exp(+-d)): all Exps, then all Lns,
                # so the ScalarE activation table swaps at most twice here
                e12s = []
                for c0 in range(0, wp, 512):
                    w = min(512, wp - c0)
                    pz = p_pw.tile([128, 512], f32, tag="pw")
                    nc.tensor.matmul(
                        pz[:2, :w], lhsT=wc3d[:, :], rhs=z2[:, c0 : c0 + w],
                        start=True, stop=True,
                    )
                    e12 = p_e.tile([2, 512], bf, tag="e12")
                    nc.scalar.activation(
                        e12[:2, :w], pz[:2, :w], AF.Exp, bias=bdd[:2, :]
                    )
                    e12s.append(e12)
                for bi, c0 in enumerate(range(0, wp, 512)):
                    w = min(512, wp - c0)
                    lt = p_mlp.tile([2, 512], f32, tag="lt")
                    nc.scalar.activation(
                        lt[:2, :w], e12s[bi][:2, :w], AF.Ln, bias=1.0
                    )
                    ng = p_mlp.tile([2, 512], f32, tag="ng")
                    nc.vector.tensor_scalar(
                        ng[:2, :w], lt[:2, :w], -1.0, None, op0=ALU.mult
                    )
                    nc.sync.dma_start(
                        out_t[gl + c0 : gl + c0 + w, 0:1], ng[0:1, :w]
                    )
                    nc.sync.dma_start(
                        out_t[gl + c0 : gl + c0 + w, 1:2], ng[1:2, :w]
                    )

            # ---------------- layer 0 B phase (startup)
            for c0 in range(0, NL, 512):
                emit_B_slice(0, c0)

            # ---------------- layers: gathers + sweeps, with next layer's B
            # slices (and layer 2's first MLP half) interleaved
            for l in range(3):
                fo = FO[l]
                # h rotates one shared buffer; all reads of the previous
                # layer's h were issued before this point.
                h_next = p_h.tile([128, NL], bf, tag="h", name=f"h{l}")
                h_tiles[l] = h_next
                next_b = [0]          # next B slice col for layer l+1
                next_part = [0]       # next MLP part (layer 2 only)
                new_ags = []          # AGs whose source slice just completed
                pend_ags = []         # (due_group, s): deferred Pool issue

                def after_group(
                    g, l=l, next_b=next_b, next_part=next_part,
                    new_ags=new_ags, pend_ags=pend_ags,
                ):
                    # B(l+1) slice j needs h(l) groups 4j..4j+3 done
                    if l < 2:
                        while (
                            next_b[0] < NL
                            and min(next_b[0] + 512, NL) <= (g + 1) * 128
                        ):
                            emit_B_slice(l + 1, next_b[0], ag_sink=new_ags)
                            next_b[0] += 512
                        while new_ags:
                            pend_ags.append((g + AG_DELAY, new_ags.pop(0)))
                        # issue AGs only once their producing chain has had
                        # AG_DELAY groups of PE headway (the Pool queue is
                        # in-order; an early AG would stall descgen)
                        while pend_ags and pend_ags[0][0] <= g:
                            _, s_ = pend_ags.pop(0)
                            fire_ag(l + 1, s_)
                    else:
                        while (
                            next_part[0] < len(MLP_THRESH)
                            and g >= MLP_THRESH[next_part[0]]
                        ):
                            emit_mlp_part(next_part[0])
                            next_part[0] += 1

                call_i = 0
                for sg in range(N_SG):
                    mb0, mb1 = sg * SG, min((sg + 1) * SG, N_MB)
                    g0, g1 = mb0 // 4, (mb1 + 3) // 4
                    # segment gathers, split into <=PIECE-chunk sub-calls
                    gtile = {}
                    stile = {}
                    while call_i < len(calls) and calls[call_i][0] == sg:
                        _, s, base, nch = calls[call_i]
                        it = p_i.tile([128, nch * 8], i16, tag="idx")
                        nc.sync.dma_start(
                            it[:], idx_h[:, base * 8 : (base + nch) * 8]
                        )
                        st = p_s.tile([128, nch * MB], bf, tag="smat")
                        nc.scalar.dma_start(
                            st[:], S_h[:, base * MB : (base + nch) * MB]
                        )
                        pieces = []
                        for p0 in range(0, nch, PIECE):
                            pn = min(PIECE, nch - p0)
                            gt = p_g.tile([K, PIECE, 128], bf, tag="gat")
                            nc.gpsimd.dma_gather(
                                out_ap=gt[:, :pn, :],
                                in_ap=tbl[l][s][:, :],
                                idxs_ap=it[:, p0 * 8 : (p0 + pn) * 8],
                                num_idxs=pn * K,
                                num_idxs_reg=pn * K,
                                elem_size=128,
                                single_packet=False,
                                queue_num=next_q(),
                            )
                            pieces.append(gt)
                        gtile[s] = (pieces, base)
                        stile[s] = st
                        call_i += 1
                    # per-group matmul sweeps
                    for g in range(g0, g1):
                        ps = p_agg.tile([128, 128], f32, tag="agg")
                        lw = min(128, NL - g * 128)
                        # self term: fetch the node-major table block back
                        # from tb_sh, scale by dinv^2, open the PSUM group
                        selft = p_sf.tile([128, 128], bf, tag="selft")
                        nc.sync.dma_start(
                            selft[:lw, :fo],
                            tb_sh[l][g * 128 : g * 128 + lw, :fo],
                        )
                        selfs = p_sf.tile([128, 128], bf, tag="selfs")
                        nc.vector.tensor_tensor(
                            selfs[:lw, :fo], selft[:lw, :fo],
                            d2t[:lw, g : g + 1].to_broadcast([lw, fo]),
                            op=ALU.mult,
                        )
                        nc.tensor.matmul(
                            ps[:fo, :lw],
                            lhsT=selfs[:lw, :fo],
                            rhs=ident[:lw, :lw],
                            start=True, stop=False,
                        )
                        gmb1 = min(4 * g + 4, mb1)
                        last_mb = gmb1 - 1
                        for mb in range(4 * g, gmb1):
                            q = mb % 4
                            colw = min(MB, NL - mb * MB)
                            for s in range(N_SEG):
                                nch_ms = cpm2[mb, s]
                                if nch_ms == 0:
                                    continue
                                pieces, gbase = gtile[s]
                                st = stile[s]
                                for k in range(nch_ms):
                                    pos = chunk_col[mb, s] + k - gbase
                                    is_last = (
                                        mb == last_mb
                                        and s == _last_seg(cpm2, mb)
                                        and k == nch_ms - 1
                                    )
                                    nc.tensor.matmul(
                                        ps[:fo, q * MB : q * MB + colw],
                                        lhsT=pieces[pos // PIECE][:, pos % PIECE, :fo],
                                        rhs=st[:, pos * MB : pos * MB + colw],
                                        start=False, stop=is_last,
                                        skip_group_check=True,
                                    )
                        nc.scalar.activation(
                            h_next[:fo, g * 128 : g * 128 + lw],
                            ps[:fo, :lw],
                            AF.Relu,
                            bias=b_tiles[l][:fo, :],
                        )
                        after_group(g)
                # flush any remaining next-layer B slices and deferred AGs
                if l < 2:
                    while next_b[0] < NL:
                        emit_B_slice(l + 1, next_b[0], ag_sink=new_ags)
                        next_b[0] += 512
                    for s_ in new_ags:
                        pend_ags.append((0, s_))
                    new_ags.clear()
                    for _, s_ in pend_ags:
                        fire_ag(l + 1, s_)
                    pend_ags.clear()
                else:
                    while next_part[0] < len(PB) - 1:
                        emit_mlp_part(next_part[0])
                        next_part[0] += 1

    nc.compile()
    return nc


# ------------------------------------------------------------------- kernel
def kernel(**inputs):
    from concourse.bass_utils import run_bass_kernel_spmd

    x = np.asarray(inputs["x"], np.float32)
    edge_index = np.asarray(inputs["edge_index"])
    struct = _preprocess(edge_index)
    nc = _build(struct)

    Wc1 = np.asarray(inputs["Wc1"], np.float32)
    Wc3 = np.asarray(inputs["Wc3"], np.float32)
    bc3 = np.asarray(inputs["bc3"], np.float32).reshape(2)
    bd = float(bc3[1] - bc3[0])
    b = [np.asarray(inputs[k], np.float32).reshape(-1, 1) for k in ("b1", "b2", "b3")]
    common = {
        "W1": np.asarray(inputs["W1"], np.float32).astype(BF16),
        "W2": np.asarray(inputs["W2"], np.float32).astype(BF16),
        "W3": np.asarray(inputs["W3"], np.float32).astype(BF16),
        "b1": b[0], "b2": b[1], "b3": b[2],
        "Wc1a": Wc1[0:128].astype(BF16),
        "Wc1b": Wc1[128:192].astype(BF16),
        "Wc2": np.asarray(inputs["Wc2"], np.float32).astype(BF16),
        "Wc3d": np.stack(
            [Wc3[:, 1] - Wc3[:, 0], Wc3[:, 0] - Wc3[:, 1]], axis=1
        ).astype(BF16),
        "bc1": np.asarray(inputs["bc1"], np.float32).reshape(-1, 1),
        "bc2": np.asarray(inputs["bc2"], np.float32).reshape(-1, 1),
        "bc3dd": np.array([[bd], [-bd]], np.float32),
    }
    in_maps = []
    for c in range(NC):
        m = dict(common)
        m["xT"] = np.ascontiguousarray(x[c * NL : (c + 1) * NL].T).astype(BF16)
        m["idx"] = struct["idx16"][c]
        m["S"] = struct["S_hbm"][c]
        m["d2nm"] = struct["d2nm"][c]
        in_maps.append(m)

    res = run_bass_kernel_spmd(nc, in_maps, core_ids=list(range(NC)))
    global LAST_RES
    LAST_RES = res
    out = np.concatenate([res.results[c]["out"] for c in range(NC)], axis=0)
    return out.astype(np.float32)


LAST_RES = None


# revision 37
# speedup vs baseline: 1.1523x; 1.1523x over previous
"""Distributed Trainium2 Bass kernel for nn_AnomalyGNN (3x GCNConv + per-graph MLP).

Strategy (8 NeuronCores, node/graph parallel):
  - Nodes block-sharded: core c owns rows [c*NL, (c+1)*NL), NL = 18750.
  - Edges sharded by destination; per core they are grouped by
    (dst miniblock of 32 nodes, source segment of 30000 nodes) and chunked
    into 128-edge chunks (row-0 padding; the chunk structure is the
    per-(mb,seg) max over cores so all 8 cores share one SPMD graph).
  - Per layer: t = h @ W for the local shard (feature-major, W stationary on
    TensorE), transposed to node-major, written to an internal DRAM shard and
    AllGathered into the full [N, 128] table. The node-major transpose blocks
    are also kept in SBUF scaled by dinv^2 (ts) for the self-loop term.
  - Message passing: MoE dma_gather (int16 idx into one of 5 segment
    sub-tables) pulls table rows; aggregation is
    agg^T[f, dst] += G_chunk^T @ S_chunk on TensorE, where S is the
    host-built [128 edges x 32 dst] matrix carrying the D^-1/2 A D^-1/2
    coefficients. Gather calls are split into <=16-chunk pieces rotating
    over the 4 SWDGE queues so descriptor generation uses all 8 GpSimd Q7
    cores; deep per-piece tile buffering keeps descgen running during the
    PSUM sweeps. Each 128-node PSUM group opens with one wide self-loop
    matmul (node-major table block DMA'd back from tb_sh, scaled by dinv^2),
    rhs=identity, then accumulates the edge chunks; epilogue relu(agg + b)
    on ScalarE emits the next feature-major h.
  - Layer l+1's table production (W-matmul, transpose, shard write and
    sub-AllGathers) is issued interleaved into layer l's PSUM sweeps so the
    collectives complete before the layer boundary; the trailing per-graph
    MLP runs in two halves, the first interleaved under layer 2's sweeps.

Compute in bf16 (f32 PSUM accumulation), I/O in f32.
"""

import numpy as np
import ml_dtypes

# ---------------------------------------------------------------- constants
N = 150000        # nodes
F = 128           # in/hidden channels
P3 = 3            # num_protocols
NC = 8            # cores
NL = N // NC      # nodes per core = 18750
GPC = NL // P3    # graphs per core = 6250
MB = 32           # dst nodes per miniblock
K = 128           # edges per chunk (= matmul contraction)
N_MB = (NL + MB - 1) // MB     # 586
SEG = 30000       # source-segment size (int16-addressable)
SUBW = 3750       # per-rank slice width per sub-AllGather; SEG = NC * SUBW
N_SEG = NL // SUBW  # 5
SG = 32           # miniblocks per supergroup (gather-call granularity)
N_SG = (N_MB + SG - 1) // SG   # 19
N_GRP = (N_MB + 3) // 4        # 147 psum groups of 128 dst nodes
FO = [128, 128, 64]            # per-layer output widths (table always 128 wide)
PIECE = 16        # max chunks per gather sub-call
import os as _os
NQ = int(_os.environ.get("KERNEL_NQ", "4"))

BF16 = ml_dtypes.bfloat16


# ------------------------------------------------------------ preprocessing
def _preprocess(edge_index):
    src = np.asarray(edge_index[0], np.int64)
    dst = np.asarray(edge_index[1], np.int64)
    deg = np.bincount(dst, minlength=N).astype(np.float64) + 1.0
    dinv = (1.0 / np.sqrt(deg)).astype(np.float32)
    norm = (dinv[src] * dinv[dst]).astype(np.float32)

    core = dst // NL
    NKEY = N_MB * N_SEG
    percore = []
    counts = np.zeros((NC, NKEY), dtype=np.int64)
    for c in range(NC):
        m = core == c
        lsrc = src[m]
        ldst = dst[m] - c * NL
        lnorm = norm[m]
        key = (ldst // MB) * N_SEG + (lsrc % NL) // SUBW
        order = np.argsort(key, kind="stable")
        lsrc, ldst, lnorm, key = lsrc[order], ldst[order], lnorm[order], key[order]
        counts[c] = np.bincount(key, minlength=NKEY)
        percore.append((lsrc, ldst, lnorm, key))

    cpm = (counts.max(axis=0) + K - 1) // K        # [NKEY] chunks per (mb, seg)
    cpm2 = cpm.reshape(N_MB, N_SEG)
    assert (cpm2.sum(axis=1) >= 1).all()

    # global chunk column order: (sg, s, mb, k)
    chunk_col = np.zeros((N_MB, N_SEG), np.int64)
    calls = []          # (sg, s, col_base, n_chunks)
    col = 0
    for sg in range(N_SG):
        mb0, mb1 = sg * SG, min((sg + 1) * SG, N_MB)
        for s in range(N_SEG):
            base = col
            for mb in range(mb0, mb1):
                chunk_col[mb, s] = col
                col += cpm2[mb, s]
            if col > base:
                calls.append((sg, s, base, col - base))
    totch = col

    idx16 = np.zeros((NC, 128, totch * 8), np.int16)
    S_hbm = np.zeros((NC, 128, totch * MB), BF16)
    for c in range(NC):
        lsrc, ldst, lnorm, key = percore[c]
        starts = np.zeros(NKEY, np.int64)
        starts[1:] = np.cumsum(counts[c])[:-1]
        pos = np.arange(len(lsrc)) - starts[key]
        colg = chunk_col[key // N_SEG, key % N_SEG] + pos // K
        lane = pos % K
        idxflat = np.zeros((totch, K), np.int64)      # default row 0 (padding)
        idxflat[colg, lane] = (lsrc // NL) * SUBW + (lsrc % NL) % SUBW
        S_all = np.zeros((totch, K, MB), np.float32)
        slot = ldst % MB
        S_all.reshape(-1)[(colg * K + lane) * MB + slot] = lnorm
        for sg, s, base, nch in calls:
            v = idxflat[base : base + nch].reshape(-1)
            a = v.reshape(nch * 8, 16).T.astype(np.int16)
            idx16[c, :, base * 8 : (base + nch) * 8] = np.tile(a, (8, 1))
        S_hbm[c] = S_all.transpose(1, 0, 2).reshape(128, totch * MB).astype(BF16)

    # d2 node-major per core: d2nm[p, g] = dinv^2 at local node g*128+p
    d2nm = np.zeros((NC, 128, N_GRP), BF16)
    for c in range(NC):
        d2 = (dinv[c * NL : (c + 1) * NL].astype(np.float64) ** 2).astype(np.float32)
        pad = np.zeros(N_GRP * 128, np.float32)
        pad[:NL] = d2
        d2nm[c] = pad.reshape(N_GRP, 128).T.astype(BF16)

    return dict(
        cpm2=cpm2,
        chunk_col=chunk_col,
        calls=calls,
        totch=totch,
        idx16=idx16,
        S_hbm=S_hbm,
        d2nm=d2nm,
    )


def _last_seg(cpm2, mb):
    for s in range(N_SEG - 1, -1, -1):
        if cpm2[mb, s] > 0:
            return s
    return 0


# ------------------------------------------------------------ graph builder
def _build(struct):
    import concourse.bass as bass
    import concourse.tile as tile
    from concourse import bacc, mybir
    from concourse.masks import make_identity

    bf = mybir.dt.bfloat16
    f32 = mybir.dt.float32
    i16 = mybir.dt.int16
    AF = mybir.ActivationFunctionType
    ALU = mybir.AluOpType

    cpm2 = struct["cpm2"]
    chunk_col = struct["chunk_col"]
    calls = struct["calls"]
    totch = struct["totch"]

    nc = bacc.Bacc(
        "TRN2", target_bir_lowering=False, debug=False, num_devices=NC,
        num_swdge_queues=NQ,
    )

    # ---- I/O
    xT = nc.dram_tensor("xT", [F, NL], bf, kind="ExternalInput")
    Wt = [
        nc.dram_tensor("W1", [F, 128], bf, kind="ExternalInput"),
        nc.dram_tensor("W2", [128, 128], bf, kind="ExternalInput"),
        nc.dram_tensor("W3", [128, 64], bf, kind="ExternalInput"),
    ]
    bt = [
        nc.dram_tensor("b1", [128, 1], f32, kind="ExternalInput"),
        nc.dram_tensor("b2", [128, 1], f32, kind="ExternalInput"),
        nc.dram_tensor("b3", [64, 1], f32, kind="ExternalInput"),
    ]
    Wc1a = nc.dram_tensor("Wc1a", [128, 128], bf, kind="ExternalInput")
    Wc1b = nc.dram_tensor("Wc1b", [64, 128], bf, kind="ExternalInput")
    Wc2 = nc.dram_tensor("Wc2", [128, 64], bf, kind="ExternalInput")
    Wc3d = nc.dram_tensor("Wc3d", [64, 2], bf, kind="ExternalInput")
    bc1 = nc.dram_tensor("bc1", [128, 1], f32, kind="ExternalInput")
    bc2 = nc.dram_tensor("bc2", [64, 1], f32, kind="ExternalInput")
    bc3dd = nc.dram_tensor("bc3dd", [2, 1], f32, kind="ExternalInput")
    idx_h = nc.dram_tensor("idx", [128, totch * 8], i16, kind="ExternalInput")
    S_h = nc.dram_tensor("S", [128, totch * MB], bf, kind="ExternalInput")
    d2_h = nc.dram_tensor("d2nm", [128, N_GRP], bf, kind="ExternalInput")
    out_t = nc.dram_tensor("out", [GPC, 2], f32, kind="ExternalOutput")

    # ---- internal DRAM (tables always 128 wide)
    tb_sh = [nc.dram_tensor(f"tbsh{l}", [NL, 128], bf) for l in range(3)]
    tbl = [
        [
            nc.dram_tensor(f"tbl{l}s{k}", [SEG, 128], bf, addr_space="Shared")
            for k in range(N_SEG)
        ]
        for l in range(3)
    ]
    RG = [list(range(NC))]

    qrr = [0]   # gather queue round-robin (DMASW lane i%8 <-> queue i%NQ)

    def next_q():
        q = qrr[0]
        qrr[0] = (q + 1) % NQ
        return q

    with tile.TileContext(nc) as tc:
        from contextlib import ExitStack

        with ExitStack() as _es:
            p_const = _es.enter_context(tc.tile_pool(name="const", bufs=1))
            p_h = _es.enter_context(tc.tile_pool(name="h", bufs=1))
            p_z = _es.enter_context(tc.tile_pool(name="z", bufs=1))
            p_mlp = _es.enter_context(tc.tile_pool(name="mlp", bufs=2))
            p_e = _es.enter_context(tc.tile_pool(name="e", bufs=7))
            p_rhs = _es.enter_context(tc.tile_pool(name="rhs", bufs=3))
            p_t = _es.enter_context(tc.tile_pool(name="tseg", bufs=3))
            p_u = _es.enter_context(tc.tile_pool(name="ust", bufs=3))
            p_sf = _es.enter_context(tc.tile_pool(name="self", bufs=6))
            p_i = _es.enter_context(tc.tile_pool(name="idx", bufs=12))
            p_s = _es.enter_context(tc.tile_pool(name="smat", bufs=9))
            p_g = _es.enter_context(tc.tile_pool(name="gat", bufs=20))
            p_pw = _es.enter_context(tc.tile_pool(name="pw", bufs=2, space="PSUM"))
            p_ptr = _es.enter_context(tc.tile_pool(name="ptr", bufs=2, space="PSUM"))
            p_agg = _es.enter_context(tc.tile_pool(name="pagg", bufs=3, space="PSUM"))
            # ---------------- persistent constants
            ident = p_const.tile([128, 128], bf, tag="ident")
            make_identity(nc, ident[:])
            w_tiles, b_tiles = [], []
            for l in range(3):
                wt = p_const.tile([128, FO[l]], bf, tag=f"w{l}")
                nc.sync.dma_start(wt[:], Wt[l][:, :])
                w_tiles.append(wt)
                btl = p_const.tile([FO[l], 1], f32, tag=f"b{l}")
                nc.sync.dma_start(btl[:], bt[l][:, :])
                b_tiles.append(btl)
            wc1a = p_const.tile([128, 128], bf, tag="wc1a")
            nc.sync.dma_start(wc1a[:], Wc1a[:, :])
            wc1b = p_const.tile([64, 128], bf, tag="wc1b")
            nc.sync.dma_start(wc1b[:], Wc1b[:, :])
            wc2 = p_const.tile([128, 64], bf, tag="wc2")
            nc.sync.dma_start(wc2[:], Wc2[:, :])
            wc3d = p_const.tile([64, 2], bf, tag="wc3d")
            nc.sync.dma_start(wc3d[:], Wc3d[:, :])
            bc1t = p_const.tile([128, 1], f32, tag="bc1")
            nc.sync.dma_start(bc1t[:], bc1[:, :])
            bc2t = p_const.tile([64, 1], f32, tag="bc2")
            nc.sync.dma_start(bc2t[:], bc2[:, :])
            bdd = p_const.tile([2, 1], f32, tag="bdd")
            nc.sync.dma_start(bdd[:], bc3dd[:, :])
            d2t = p_const.tile([128, N_GRP], bf, tag="d2")
            nc.sync.dma_start(d2t[:], d2_h[:, :])

            h_tiles = [None, None, None]
            ag_state = [0, 0, 0]   # next sub-AllGather per layer
            AG_DELAY = 7           # groups between AG readiness and issue

            def fire_ag(l, s):
                nc.gpsimd.collective_compute(
                    "AllGather",
                    ALU.bypass,
                    replica_groups=RG,
                    ins=[tb_sh[l][s * SUBW : (s + 1) * SUBW, :].opt()],
                    outs=[tbl[l][s].ap().opt()],
                )

            # ---- B phase: one 512-col slice of t = h @ W for layer l.
            # Emits matmul, transpose, shard write; fires (or queues into
            # ag_sink) any sub-AllGathers whose source slice completed. The
            # self-loop term is re-fetched from tb_sh per PSUM group during
            # the sweep (no persistent ts).
            def emit_B_slice(l, c0, ag_sink=None):
                fi = F if l == 0 else FO[l - 1]
                fo = FO[l]
                w = min(512, NL - c0)
                if l == 0:
                    rhs_t = p_rhs.tile([F, 512], bf)
                    nc.sync.dma_start(rhs_t[:, :w], xT[:, c0 : c0 + w])
                    rhs_ap = rhs_t[:, :w]
                else:
                    rhs_ap = h_tiles[l - 1][:fi, c0 : c0 + w]
                pt = p_pw.tile([128, 512], f32, tag="pw")
                nc.tensor.matmul(
                    pt[:fo, :w], lhsT=w_tiles[l][:fi, :fo], rhs=rhs_ap,
                    start=True, stop=True,
                )
                tseg = p_t.tile([128, 512], bf)
                nc.vector.tensor_copy(tseg[:fo, :w], pt[:fo, :w])
                ust = p_u.tile([128, 4, 128], bf)
                nbl = (w + 127) // 128
                for sb in range(nbl):
                    sw = min(128, w - sb * 128)
                    ptt = p_ptr.tile([128, 128], bf, tag="ptr")
                    nc.tensor.transpose(
                        ptt[:sw, :fo],
                        tseg[:fo, sb * 128 : sb * 128 + sw],
                        ident[:fo, :fo],
                    )
                    nc.vector.tensor_copy(ust[:sw, sb, :fo], ptt[:sw, :fo])
                nfull = w // 128
                if nfull:
                    dst_ap = tb_sh[l][c0 : c0 + nfull * 128, :].rearrange(
                        "(j p) f -> p j f", p=128
                    )
                    nc.sync.dma_start(dst_ap, ust[:, :nfull, :])
                if w % 128:
                    tw = w % 128
                    nc.sync.dma_start(
                        tb_sh[l][c0 + nfull * 128 : c0 + w, :],
                        ust[:tw, nfull, :],
                    )
                while ag_state[l] < N_SEG and c0 + w >= (ag_state[l] + 1) * SUBW:
                    s = ag_state[l]
                    if ag_sink is None:
                        fire_ag(l, s)
                    else:
                        ag_sink.append(s)
                    ag_state[l] += 1

            # ---- MLP parts: graph ranges fired as their h columns complete.
            # 2-class log_softmax via softplus: out0 = -softplus(d),
            # out1 = -softplus(-d), d = z2.(wc3[:,1]-wc3[:,0]) + (bc3[1]-bc3[0])
            PB = [0, 3125, 4688, GPC]
            MLP_THRESH = [73, 109]     # sweep group gating parts 0 and 1
            PWID = 3125

            def emit_mlp_part(p):
                h_cur = h_tiles[2]
                gl, gr = PB[p], PB[p + 1]
                wp = gr - gl
                zA = p_z.tile([128, PWID], bf, tag="zA")
                zB = p_z.tile([64, PWID], bf, tag="zB")
                z1 = p_z.tile([128, PWID], bf, tag="z1")
                z2 = p_z.tile([64, PWID], bf, tag="z2")
                nc.vector.tensor_copy(zA[0:64, :wp], h_cur[:64, 3 * gl + 0 : 3 * gr : 3])
                nc.vector.tensor_copy(zA[64:128, :wp], h_cur[:64, 3 * gl + 1 : 3 * gr : 3])
                nc.vector.tensor_copy(zB[:, :wp], h_cur[:64, 3 * gl + 2 : 3 * gr : 3])
                for c0 in range(0, wp, 512):
                    w = min(512, wp - c0)
                    pz = p_pw.tile([128, 512], f32, tag="pw")
                    nc.tensor.matmul(
                        pz[:, :w], lhsT=wc1a[:], rhs=zA[:, c0 : c0 + w],
                        start=True, stop=False,
                    )
                    nc.tensor.matmul(
                        pz[:, :w], lhsT=wc1b[:], rhs=zB[:, c0 : c0 + w],
                        start=False, stop=True,
                    )
                    nc.scalar.activation(
                        z1[:, c0 : c0 + w], pz[:, :w], AF.Relu, bias=bc1t[:, :]
                    )
                for c0 in range(0, wp, 512):
                    w = min(512, wp - c0)
                    pz = p_pw.tile([128, 512], f32, tag="pw")
                    nc.tensor.matmul(
                        pz[:64, :w], lhsT=wc2[:], rhs=z1[:, c0 : c0 + w],
                        start=True, stop=True,
                    )
                    nc.scalar.activation(
                        z2[:, c0 : c0 + w], pz[:64, :w], AF.Relu, bias=bc2t[:, :]
                    )
                # log_softmax via -ln(1+exp(+-d)): all Exps, then all Lns,
                # so the ScalarE activation table swaps at most twice here
                e12s = []
                for c0 in range(0, wp, 512):
                    w = min(512, wp - c0)
                    pz = p_pw.tile([128, 512], f32, tag="pw")
                    nc.tensor.matmul(
                        pz[:2, :w], lhsT=wc3d[:, :], rhs=z2[:, c0 : c0 + w],
                        start=True, stop=True,
                    )
                    e12 = p_e.tile([2, 512], bf, tag="e12")
                    nc.scalar.activation(
                        e12[:2, :w], pz[:2, :w], AF.Exp, bias=bdd[:2, :]
                    )
                    e12s.append(e12)
                for bi, c0 in enumerate(range(0, wp, 512)):
                    w = min(512, wp - c0)
                    lt = p_mlp.tile([2, 512], f32, tag="lt")
                    nc.scalar.activation(
                        lt[:2, :w], e12s[bi][:2, :w], AF.Ln, bias=1.0
                    )
                    ng = p_mlp.tile([2, 512], f32, tag="ng")
                    nc.vector.tensor_scalar(
                        ng[:2, :w], lt[:2, :w], -1.0, None, op0=ALU.mult
                    )
                    nc.sync.dma_start(
                        out_t[gl + c0 : gl + c0 + w, 0:1], ng[0:1, :w]
                    )
                    nc.sync.dma_start(
                        out_t[gl + c0 : gl + c0 + w, 1:2], ng[1:2, :w]
                    )

            # ---------------- layer 0 B phase (startup)
            for c0 in range(0, NL, 512):
                emit_B_slice(0, c0)

            # ---------------- layers: gathers + sweeps, with next layer's B
            # slices (and layer 2's first MLP half) interleaved
            for l in range(3):
                fo = FO[l]
                # h rotates one shared buffer; all reads of the previous
                # layer's h were issued before this point.
                h_next = p_h.tile([128, NL], bf, tag="h", name=f"h{l}")
                h_tiles[l] = h_next
                next_b = [0]          # next B slice col for layer l+1
                next_part = [0]       # next MLP part (layer 2 only)
                new_ags = []          # AGs whose source slice just completed
                pend_ags = []         # (due_group, s): deferred Pool issue

                def after_group(
                    g, l=l, next_b=next_b, next_part=next_part,
                    new_ags=new_ags, pend_ags=pend_ags,
                ):
                    # B(l+1) slice j needs h(l) groups 4j..4j+3 done
                    if l < 2:
                        while (
                            next_b[0] < NL
                            and min(next_b[0] + 512, NL) <= (g + 1) * 128
                        ):
                            emit_B_slice(l + 1, next_b[0], ag_sink=new_ags)
                            next_b[0] += 512
                        while new_ags:
                            pend_ags.append((g + AG_DELAY, new_ags.pop(0)))
                        # issue AGs only once their producing chain has had
                        # AG_DELAY groups of PE headway (the Pool queue is
                        # in-order; an early AG would stall descgen)
                        while pend_ags and pend_ags[0][0] <= g:
                            _, s_ = pend_ags.pop(0)
                            fire_ag(l + 1, s_)
                    else:
                        while (
                            next_part[0] < len(MLP_THRESH)
                            and g >= MLP_THRESH[next_part[0]]
                        ):
                            emit_mlp_part(next_part[0])
                            next_part[0] += 1

                call_i = 0
                for sg in range(N_SG):
                    mb0, mb1 = sg * SG, min((sg + 1) * SG, N_MB)
                    g0, g1 = mb0 // 4, (mb1 + 3) // 4
                    # segment gathers, split into <=PIECE-chunk sub-calls
                    gtile = {}
                    stile = {}
                    while call_i < len(calls) and calls[call_i][0] == sg:
                        _, s, base, nch = calls[call_i]
                        it = p_i.tile([128, nch * 8], i16, tag="idx")
                        nc.sync.dma_start(
                            it[:], idx_h[:, base * 8 : (base + nch) * 8]
                        )
                        st = p_s.tile([128, nch * MB], bf, tag="smat")
                        nc.scalar.dma_start(
                            st[:], S_h[:, base * MB : (base + nch) * MB]
                        )
                        pieces = []
                        for p0 in range(0, nch, PIECE):
                            pn = min(PIECE, nch - p0)
                            gt = p_g.tile([K, PIECE, 128], bf, tag="gat")
                            nc.gpsimd.dma_gather(
                                out_ap=gt[:, :pn, :],
                                in_ap=tbl[l][s][:, :],
                                idxs_ap=it[:, p0 * 8 : (p0 + pn) * 8],
                                num_idxs=pn * K,
                                num_idxs_reg=pn * K,
                                elem_size=128,
                                single_packet=False,
                                queue_num=next_q(),
                            )
                            pieces.append(gt)
                        gtile[s] = (pieces, base)
                        stile[s] = st
                        call_i += 1
                    # per-group matmul sweeps
                    for g in range(g0, g1):
                        ps = p_agg.tile([128, 128], f32, tag="agg")
                        lw = min(128, NL - g * 128)
                        # self term: fetch the node-major table block back
                        # from tb_sh, scale by dinv^2, open the PSUM group
                        selft = p_sf.tile([128, 128], bf, tag="selft")
                        nc.sync.dma_start(
                            selft[:lw, :fo],
                            tb_sh[l][g * 128 : g * 128 + lw, :fo],
                        )
                        selfs = p_sf.tile([128, 128], bf, tag="selfs")
                        nc.vector.tensor_tensor(
                            selfs[:lw, :fo], selft[:lw, :fo],
                            d2t[:lw, g : g + 1].to_broadcast([lw, fo]),
                            op=ALU.mult,
                        )
                        nc.tensor.matmul(
                            ps[:fo, :lw],
                            lhsT=selfs[:lw, :fo],
                            rhs=ident[:lw, :lw],
                            start=True, stop=False,
                        )
                        gmb1 = min(4 * g + 4, mb1)
                        last_mb = gmb1 - 1
                        for mb in range(4 * g, gmb1):
                            q = mb % 4
                            colw = min(MB, NL - mb * MB)
                            for s in range(N_SEG):
                                nch_ms = cpm2[mb, s]
                                if nch_ms == 0:
                                    continue
                                pieces, gbase = gtile[s]
                                st = stile[s]
                                for k in range(nch_ms):
                                    pos = chunk_col[mb, s] + k - gbase
                                    is_last = (
                                        mb == last_mb
                                        and s == _last_seg(cpm2, mb)
                                        and k == nch_ms - 1
                                    )
                                    nc.tensor.matmul(
                                        ps[:fo, q * MB : q * MB + colw],
                                        lhsT=pieces[pos // PIECE][:, pos % PIECE, :fo],
                                        rhs=st[:, pos * MB : pos * MB + colw],
                                        start=False, stop=is_last,
                                        skip_group_check=True,
                                    )
                        nc.scalar.activation(
                            h_next[:fo, g * 128 : g * 128 + lw],
                            ps[:fo, :lw],
                            AF.Relu,
                            bias=b_tiles[l][:fo, :],
                        )
                        after_group(g)
                # flush any remaining next-layer B slices and deferred AGs
                if l < 2:
                    while next_b[0] < NL:
                        emit_B_slice(l + 1, next_b[0], ag_sink=new_ags)
                        next_b[0] += 512
                    for s_ in new_ags:
                        pend_ags.append((0, s_))
                    new_ags.clear()
                    for _, s_ in pend_ags:
                        fire_ag(l + 1, s_)
                    pend_ags.clear()
                else:
                    while next_part[0] < len(PB) - 1:
                        emit_mlp_part(next_part[0])
                        next_part[0] += 1

    nc.compile()
    return nc


# ------------------------------------------------------------------- kernel
def kernel(**inputs):
    from concourse.bass_utils import run_bass_kernel_spmd

    x = np.asarray(inputs["x"], np.float32)
    edge_index = np.asarray(inputs["edge_index"])
    struct = _preprocess(edge_index)
    nc = _build(struct)

    Wc1 = np.asarray(inputs["Wc1"], np.float32)
    Wc3 = np.asarray(inputs["Wc3"], np.float32)
    bc3 = np.asarray(inputs["bc3"], np.float32).reshape(2)
    bd = float(bc3[1] - bc3[0])
    b = [np.asarray(inputs[k], np.float32).reshape(-1, 1) for k in ("b1", "b2", "b3")]
    common = {
        "W1": np.asarray(inputs["W1"], np.float32).astype(BF16),
        "W2": np.asarray(inputs["W2"], np.float32).astype(BF16),
        "W3": np.asarray(inputs["W3"], np.float32).astype(BF16),
        "b1": b[0], "b2": b[1], "b3": b[2],
        "Wc1a": Wc1[0:128].astype(BF16),
        "Wc1b": Wc1[128:192].astype(BF16),
        "Wc2": np.asarray(inputs["Wc2"], np.float32).astype(BF16),
        "Wc3d": np.stack(
            [Wc3[:, 1] - Wc3[:, 0], Wc3[:, 0] - Wc3[:, 1]], axis=1
        ).astype(BF16),
        "bc1": np.asarray(inputs["bc1"], np.float32).reshape(-1, 1),
        "bc2": np.asarray(inputs["bc2"], np.float32).reshape(-1, 1),
        "bc3dd": np.array([[bd], [-bd]], np.float32),
    }
    in_maps = []
    for c in range(NC):
        m = dict(common)
        m["xT"] = np.ascontiguousarray(x[c * NL : (c + 1) * NL].T).astype(BF16)
        m["idx"] = struct["idx16"][c]
        m["S"] = struct["S_hbm"][c]
        m["d2nm"] = struct["d2nm"][c]
        in_maps.append(m)

    res = run_bass_kernel_spmd(nc, in_maps, core_ids=list(range(NC)))
    global LAST_RES
    LAST_RES = res
    out = np.concatenate([res.results[c]["out"] for c in range(NC)], axis=0)
    return out.astype(np.float32)


LAST_RES = None
